# revision 1
# baseline (speedup 1.0000x reference)
"""PointsFusion2 Bass kernel builder + host prep + numpy mirror for verification."""
import numpy as np
import concourse.bass as bass
import concourse.tile as tile
from concourse import bacc, mybir

F32 = mybir.dt.float32
U16 = mybir.dt.uint16
U32 = mybir.dt.uint32
AF = mybir.ActivationFunctionType
ALU = mybir.AluOpType
AX = mybir.AxisListType
EPS = 1e-5

M = 4096          # reference points per set
S = 24            # top-k slots kept per set (k0,k1 <= 24)
K = 32            # total neighbors


def _sigma(s, j):
    """slot used by concat-column j for set s (set1 fills from the tail)."""
    return j if s == 0 else 31 - j


# ---------------------------------------------------------------- host prep
def host_prep(points0, points1, k, weighted_t, perm, w1, b1, gn1_w, gn1_b,
              w2, b2, gn2_w, gn2_b, n_tiles=32):
    """Build per-core input dicts (one per sample) for the SPMD program."""
    B = points0.shape[0]
    N = 128 * n_tiles
    w1n = np.asarray(w1, np.float32)          # [32, 4]
    w2n = np.asarray(w2, np.float32)          # [64, 32]
    # conv1 block weights: w1big[s, jgrp, (c,sig), (o,jlo)]
    w1big = np.zeros((2, 8, 128, 128), np.float32)
    for s in range(2):
        for jg in range(8):
            for jlo in range(4):
                j = jg * 4 + jlo
                sig = _sigma(s, j)
                for o in range(32):
                    for c in range(4):
                        w1big[s, jg, c * 32 + sig, o * 4 + jlo] = w1n[o, c]
    # conv2 block weights: w2big[jlo, (o,jlo'), o2]
    w2big = np.zeros((4, 128, 64), np.float32)
    for jlo in range(4):
        for o in range(32):
            for o2 in range(64):
                w2big[jlo, o * 4 + jlo, o2] = w2n[o2, o]
    G1p = np.zeros((128, 4), np.float32)     # (jlo,o) -> group o//8
    for p in range(128):
        G1p[p, (p % 32) // 8] = 1.0
    G2 = np.zeros((64, 8), np.float32)
    for o in range(64):
        G2[o, o // 8] = 1.0
    b1a = np.asarray(b1, np.float32)
    b2a = np.asarray(b2, np.float32)
    # per-(o,jlo) replicated conv1 consts; plain conv2 consts
    cpk2 = np.zeros((128, 8), np.float32)
    cpk2[:, 0] = np.tile(b1a, 4)
    cpk2[:, 1] = np.tile(np.asarray(gn1_w, np.float32), 4)
    cpk2[:, 2] = np.tile(np.asarray(gn1_b, np.float32), 4)
    cpk2[:64, 3] = b2a
    cpk2[:64, 4] = np.asarray(gn2_w, np.float32)
    cpk2[:64, 5] = np.asarray(gn2_b, np.float32)
    G1g = np.zeros((4, 32), np.float32)      # unused rows pad to [4,128] bcast
    hc1 = np.stack([8.0 * (b1a.reshape(4, 8).sum(1)),
                    8.0 * ((b1a * b1a).reshape(4, 8).sum(1))], 1) / 8.0
    # NOTE: hc entries = sum over channels in group of b1 / b1^2
    hc1 = np.stack([b1a.reshape(4, 8).sum(1),
                    (b1a * b1a).reshape(4, 8).sum(1)], 1)
    hc2 = np.stack([b2a.reshape(8, 8).sum(1),
                    (b2a * b2a).reshape(8, 8).sum(1)], 1)
    G1Tp = G1p.T.copy()                      # [4, 128]
    G2T = G2.T.copy()                        # [8, 64]
    ident = np.eye(128, dtype=np.float32)
    ins = []
    for i in range(B):
        p0 = np.asarray(points0[i], np.float32)   # [3, 4096]
        p1 = np.asarray(points1[i], np.float32)
        wt = float(np.asarray(weighted_t[i, 0]))
        N0 = int(4096 * wt)
        k0 = int(K * wt)
        k1 = K - k0
        pm = np.asarray(perm[i])
        newp = np.concatenate([p0[:, pm[0, :N0]], p1[:, pm[1, :4096 - N0]]],
                              axis=1)[:, :N]      # [3, N]
        qaug = np.concatenate([2.0 * newp, np.ones((1, N), np.float32)], 0)
        paug = np.stack([
            np.concatenate([p0, -np.sum(p0 * p0, 0, keepdims=True)], 0),
            np.concatenate([p1, -np.sum(p1 * p1, 0, keepdims=True)], 0)])
        table0 = p0.T.copy()                                                 # [M,3]
        table1 = p1.T.copy()
        qxyz = np.concatenate([newp.T, np.sum(newp * newp, 0)[:, None]], 1)  # [N,4]
        sig = np.arange(K, dtype=np.float32)
        maskf = np.stack([
            np.repeat((sig < k0)[None, :], 128, 0),
            np.repeat((sig < k1)[None, :], 128, 0)]).astype(np.float32)      # [2,128,32]
        # per-(c,sigma) partition mask for FT eviction
        sigp = np.arange(128) % 32
        maskp = np.stack([(sigp < k0), (sigp < k1)], 1).astype(np.float32)   # [128,2]
        ins.append(dict(qaug=qaug, paug=paug, table0=table0, table1=table1,
                        qxyz=qxyz,
                        maskf=maskf, maskp=maskp,
                        w1T=w1n.T.copy(), w2T=w2n.T.copy(),
                        cpk2=cpk2, hc1=hc1, hc2=hc2, G1p=G1p, G1Tp=G1Tp,
                        G2=G2, G2T=G2T))
    return ins


# ---------------------------------------------------------------- device program
def build(nc, n_tiles=32):
    NT = n_tiles
    N = 128 * NT

    def din(name, shape):
        return nc.dram_tensor(name, shape, F32, kind="ExternalInput").ap()

    d = dict(
        qaug=din("qaug", [4, N]),
        paug=din("paug", [2, 4, M]),
        table0=din("table0", [M, 3]),
        table1=din("table1", [M, 3]),
        qxyz=din("qxyz", [N, 4]),
        maskf=din("maskf", [2, 128, K]),
        maskp=din("maskp", [128, 2]),
        w1T=din("w1T", [4, 32]),
        w2T=din("w2T", [32, 64]),
        cpk2=din("cpk2", [128, 8]),
        hc1=din("hc1", [4, 2]),
        hc2=din("hc2", [8, 2]),
        G1p=din("G1p", [128, 4]),
        G1Tp=din("G1Tp", [4, 128]),
        G2=din("G2", [64, 8]),
        G2T=din("G2T", [8, 64]),
    )
    out_d = nc.dram_tensor("out", [N, 3], F32, kind="ExternalOutput").ap()

    with tile.TileContext(nc) as tc:
        _build_tc(nc, tc, NT, N, d, out_d)
    nc.compile()
    return nc


def _ap(t, offset, dims):
    """Flat-address AP (use for DRAM tensors)."""
    base = t[:] if not isinstance(t, bass.AP) else t
    return bass.AP(base.tensor, offset, dims)


def sap(t, coff, freedims, p0=0, pcnt=128, pstep=1):
    """SBUF AP: partitions [p0, p0+pstep*pcnt), column offset coff, free dims."""
    base = t[:] if not isinstance(t, bass.AP) else t
    pitch = base.ap[0][0]
    return bass.AP(base.tensor, p0 * pitch + coff,
                   [[pstep * pitch, pcnt]] + freedims)


def _build_tc(nc, tc, NT, N, d, out_d):
    import contextlib
    ctx = contextlib.ExitStack()
    pool = ctx.enter_context(tc.tile_pool(name="persist", bufs=1))

    # ---------------- persistent SBUF loads
    qaug = pool.tile([4, N], F32)
    nc.sync.dma_start(qaug[:], d["qaug"][:])
    paug = pool.tile([4, 2 * M], F32)    # free = (set, m)
    nc.sync.dma_start(
        sap(paug, 0, [[M, 2], [1, M]], pcnt=4),
        _ap(d["paug"], 0, [[M, 4], [4 * M, 2], [1, M]]))
    qxyz = pool.tile([128, NT * 4], F32)
    nc.sync.dma_start(qxyz[:], _ap(d["qxyz"], 0, [[4, 128], [512, NT], [1, 4]]))
    maskf = pool.tile([128, 2 * K], F32)  # free = (set, sigma)
    nc.sync.dma_start(
        sap(maskf, 0, [[K, 2], [1, K]]),
        _ap(d["maskf"], 0, [[K, 128], [128 * K, 2], [1, K]]))
    maskp = pool.tile([128, 2], F32)
    nc.sync.dma_start(maskp[:], d["maskp"][:])
    w1big = pool.tile([128, 16 * 128], F32)   # free = (s*8+jgrp, jlo*32+o)
    nc.vector.memset(w1big[:], 0)
    for s in range(2):
        for jg in range(8):
            for jlo in range(4):
                sg = _sigma(s, jg * 4 + jlo)
                for c in range(4):
                    nc.sync.dma_start(
                        sap(w1big, (s * 8 + jg) * 128 + jlo * 32, [[1, 32]],
                            p0=c * 32 + sg, pcnt=1),
                        _ap(d["w1T"], c * 32, [[32, 1], [1, 32]]))
    w2big = pool.tile([128, 4 * 64], F32)     # free = (jlo, o2)
    nc.vector.memset(w2big[:], 0)
    for jlo in range(4):
        nc.sync.dma_start(
            sap(w2big, jlo * 64, [[1, 64]], p0=jlo * 32, pcnt=32),
            _ap(d["w2T"], 0, [[64, 32], [1, 64]]))
    cpk2 = pool.tile([128, 8], F32)
    nc.sync.dma_start(cpk2[:], d["cpk2"][:])
    hc1 = pool.tile([4, 2], F32)
    nc.sync.dma_start(hc1[:], d["hc1"][:])
    hc2 = pool.tile([8, 2], F32)
    nc.sync.dma_start(hc2[:], d["hc2"][:])
    G1p = pool.tile([128, 4], F32)
    nc.sync.dma_start(G1p[:], d["G1p"][:])
    G1Tp = pool.tile([4, 128], F32)
    nc.sync.dma_start(G1Tp[:], d["G1Tp"][:])
    G2 = pool.tile([64, 8], F32)
    nc.sync.dma_start(G2[:], d["G2"][:])
    G2T = pool.tile([8, 64], F32)
    nc.sync.dma_start(G2T[:], d["G2T"][:])
    ident = pool.tile([128, 128], F32)
    ones = pool.tile([128, 1], F32)
    nc.vector.memset(ones[:], 1.0)
    nc.gpsimd.affine_select(ident[:], ones[:].to_broadcast([128, 128]),
                            [[1, 128]], ALU.is_equal, 0.0,
                            base=0, channel_multiplier=-1)

    # persistent intermediates
    ftall = [pool.tile([128, N], F32, name=f"ftall{s}", tag=f"ftall{s}")
             for s in range(2)]                       # [(c,sig), (t,q)]
    resi = [pool.tile([128, NT * 3 * S], F32, name=f"resi{s}", tag=f"resi{s}")
            for s in range(2)]
    WSZ = 512 if N >= 512 else N                      # query window size
    NW = N // WSZ
    s1buf = pool.tile([128, 8 * NW], F32)             # (jgrp, win)
    sq1buf = pool.tile([128, 8 * NW], F32)
    s2buf = pool.tile([64, 8 * NW * 4], F32)          # (jgrp, win, jlo)
    sq2buf = pool.tile([64, 8 * NW * 4], F32)
    A1 = pool.tile([128, 1], F32, tag="A1")
    B1 = pool.tile([128, 1], F32, tag="B1")
    A2 = pool.tile([64, 1], F32, tag="A2")
    B2 = pool.tile([64, 1], F32, tag="B2")
    scall = pool.tile([128, NT * K], F32)             # scores, col t*32+j

    # =================== PHASE A: KNN + features ===================
    with (tc.tile_pool(name="tsb", bufs=2) as tsb_pool,
          tc.tile_pool(name="dps", bufs=3, space="PSUM") as dps_pool,
          tc.tile_pool(name="ftp", bufs=2, space="PSUM") as ftp_pool,
          tc.tile_pool(name="sm", bufs=4) as sm_pool,
          tc.tile_pool(name="gth", bufs=2) as gth_pool):
        for s in range(2):
            tabd = d["table0"] if s == 0 else d["table1"]
            for t in range(NT):
                ts = tsb_pool.tile([128, M], F32, tag="ts")
                for c4 in range(4):
                    ps = dps_pool.tile([128, 1024], F32, tag="dps")
                    for h in range(2):
                        nc.tensor.matmul(
                            ps[:, h * 512:(h + 1) * 512],
                            qaug[:, t * 128:(t + 1) * 128],
                            paug[:, s * M + c4 * 1024 + h * 512:
                                 s * M + c4 * 1024 + (h + 1) * 512],
                            start=True, stop=True)
                    nc.scalar.copy(ts[:, c4 * 1024:(c4 + 1) * 1024], ps[:])
                # top-24 (descending score == ascending distance)
                V = sm_pool.tile([128, S], F32, tag="V")
                I = sm_pool.tile([128, S], U32, tag="I")
                for r in range(3):
                    nc.vector.max(V[:, r * 8:(r + 1) * 8], ts[:])
                    nc.vector.max_index(I[:, r * 8:(r + 1) * 8],
                                        V[:, r * 8:(r + 1) * 8], ts[:])
                    if r < 2:
                        nc.vector.match_replace(ts[:], V[:, r * 8:(r + 1) * 8],
                                                ts[:], -1e30)
                nn = gth_pool.tile([128, 3 * S], F32, tag="nn")  # (sigma, c)
                for sg in range(S):
                    nc.gpsimd.indirect_dma_start(
                        nn[:, sg * 3:(sg + 1) * 3], None, tabd[:],
                        bass.IndirectOffsetOnAxis(ap=I[:, sg:sg + 1], axis=0))
                # F tile: [128, (c4, sigma32)] ; c=0..2 resi, c=3 dist
                F = gth_pool.tile([128, 128], F32, tag="F")
                nc.vector.memset(sap(F, S, [[32, 4], [1, 32 - S]]), 0)
                nc.vector.tensor_copy(
                    sap(F, 0, [[32, 3], [1, S]]),
                    sap(nn, 0, [[1, 3], [3, S]]))
                # resi = p - q
                nc.vector.tensor_tensor(
                    sap(F, 0, [[32, 3], [1, S]]),
                    sap(F, 0, [[32, 3], [1, S]]),
                    sap(qxyz, t * 4, [[1, 3], [0, S]]),
                    ALU.subtract)
                # dist = sqrt(max(qsq - V, 0))
                d2n = sm_pool.tile([128, S], F32, tag="d2n")
                nc.vector.tensor_scalar(d2n[:], V[:],
                                        qxyz[:, t * 4 + 3:t * 4 + 4],
                                        0.0, ALU.subtract, ALU.min)
                nc.scalar.activation(F[:, 96:96 + S], d2n[:], AF.Sqrt,
                                     scale=-1.0)
                # save resi for final aggregation (unmasked; weights are masked)
                nc.vector.tensor_copy(
                    resi[s][:, t * 3 * S:(t + 1) * 3 * S],
                    sap(F, 0, [[32, 3], [1, S]]))
                # transpose -> FT [(c,sigma), q]; evict with slot-validity mask
                ftps = ftp_pool.tile([128, 128], F32, tag="ftps")
                nc.tensor.transpose(ftps[:], F[:], ident[:])
                nc.scalar.activation(ftall[s][:, t * 128:(t + 1) * 128],
                                     ftps[:], AF.Identity,
                                     scale=maskp[:, s:s + 1])

    # =================== conv helpers ===================
    def conv1(x1ps, jg, w0):
        nc.tensor.matmul(x1ps[:], w1big[:, jg * 128:(jg + 1) * 128],
                         ftall[0][:, w0:w0 + WSZ], start=True, stop=False)
        nc.tensor.matmul(x1ps[:], w1big[:, (8 + jg) * 128:(9 + jg) * 128],
                         ftall[1][:, w0:w0 + WSZ], start=False, stop=True)

    # =================== PHASE P1: conv1 stats ===================
    with (tc.tile_pool(name="x1p1", bufs=2, space="PSUM") as x1_pool,
          tc.tile_pool(name="scr1", bufs=2) as scr_pool):
        for jg in range(8):
            for w in range(NW):
                idx = jg * NW + w
                x1ps = x1_pool.tile([128, WSZ], F32, tag="x1ps")
                conv1(x1ps, jg, w * WSZ)
                scr = scr_pool.tile([128, WSZ], F32, tag="scr")
                nc.scalar.activation(scr[:], x1ps[:], AF.Square,
                                     accum_out=sq1buf[:, idx:idx + 1])
                scr1 = scr_pool.tile([128, WSZ], F32, tag="scr1")
                nc.scalar.activation(scr1[:], x1ps[:], AF.Identity,
                                     accum_out=s1buf[:, idx:idx + 1])

    # =================== stats finalize (GN affine params) ===================
    def gn_finalize(C, ng, sbuf, sqbuf, Gm, GmT, hc, bcol, wcol, bcol2, A, B,
                    stp_pool, stps_pool):
        n_per = float(N * K)
        Np = 8.0 * n_per
        st = stp_pool.tile([C, 4], F32, name=f"st{C}", tag=f"st{C}")
        nc.vector.tensor_reduce(st[:, 0:1], sbuf[:], AX.X, ALU.add)
        nc.vector.tensor_reduce(st[:, 1:2], sqbuf[:], AX.X, ALU.add)
        nc.vector.tensor_mul(st[:, 2:3], st[:, 0:1], cpk2[0:C, bcol:bcol + 1])
        nc.vector.memset(st[:, 3:4], 0)
        gps = stps_pool.tile([ng, 4], F32, name=f"gps{ng}", tag=f"gps{ng}")
        nc.tensor.matmul(gps[:], Gm[:], st[:], start=True, stop=True)
        gs = stp_pool.tile([ng, 8], F32, name=f"gs{ng}", tag=f"gs{ng}")
        nc.scalar.copy(gs[:, 0:4], gps[:])
        bs_scale = 2.0 / Np
        s_scale = 1.0 / Np
        # mu' = S*s_scale + hc0*(n_per/Np); E2 = Q*s_scale + BS*bs_scale + hc1*(n_per/Np)
        nc.vector.tensor_scalar(gs[:, 4:5], hc[:, 0:1], n_per / Np, None,
                                ALU.mult)
        nc.vector.tensor_scalar(gs[:, 5:6], gs[:, 0:1], s_scale, gs[:, 4:5],
                                ALU.mult, ALU.add)          # mu'
        nc.vector.tensor_scalar(gs[:, 6:7], gs[:, 2:3], bs_scale, None,
                                ALU.mult)
        nc.vector.tensor_scalar(gs[:, 7:8], hc[:, 1:2], n_per / Np,
                                gs[:, 6:7], ALU.mult, ALU.add)
        nc.vector.tensor_scalar(gs[:, 7:8], gs[:, 1:2], s_scale, gs[:, 7:8],
                                ALU.mult, ALU.add)          # E2
        nc.vector.tensor_mul(gs[:, 4:5], gs[:, 5:6], gs[:, 5:6])
        nc.vector.tensor_sub(gs[:, 4:5], gs[:, 7:8], gs[:, 4:5])   # var
        nc.vector.tensor_scalar(gs[:, 4:5], gs[:, 4:5], EPS, None, ALU.add)
        nc.scalar.activation(gs[:, 4:5], gs[:, 4:5], AF.Sqrt)
        nc.vector.reciprocal(gs[:, 4:5], gs[:, 4:5])               # rs
        pps = stps_pool.tile([C, 2], F32, name=f"pps{C}", tag=f"pps{C}")
        nc.tensor.matmul(pps[:], GmT[:],
                         sap(gs, 5, [[-1, 2]], pcnt=ng), start=True, stop=True)
        po = stp_pool.tile([C, 2], F32, name=f"po{C}", tag=f"po{C}")
        nc.scalar.copy(po[:], pps[:])    # col0 = mu', col1 = rs
        nc.vector.tensor_mul(A[:], cpk2[0:C, wcol:wcol + 1], po[:, 1:2])
        nc.vector.tensor_sub(B[:], cpk2[0:C, bcol:bcol + 1], po[:, 0:1])
        nc.vector.tensor_mul(B[:], B[:], A[:])
        nc.vector.tensor_add(B[:], B[:], cpk2[0:C, bcol2:bcol2 + 1])

    with (tc.tile_pool(name="stp", bufs=1) as stp_pool,
          tc.tile_pool(name="stps", bufs=1, space="PSUM") as stps_pool):
        gn_finalize(128, 4, s1buf, sq1buf, G1p, G1Tp, hc1, 0, 1, 2, A1, B1,
                    stp_pool, stps_pool)

    # =================== PHASE P2: conv2 stats ===================
    with (tc.tile_pool(name="x1p2", bufs=2, space="PSUM") as x1_pool,
          tc.tile_pool(name="x2p2", bufs=2, space="PSUM") as x2_pool,
          tc.tile_pool(name="x1r2", bufs=2) as x1r_pool,
          tc.tile_pool(name="scr2", bufs=2) as scr_pool):
        for jg in range(8):
            for w in range(NW):
                x1ps = x1_pool.tile([128, WSZ], F32, tag="x1ps")
                conv1(x1ps, jg, w * WSZ)
                x1r = x1r_pool.tile([128, WSZ], F32, tag="x1r")
                nc.scalar.activation(x1r[:], x1ps[:], AF.Relu,
                                     bias=B1[:], scale=A1[:])
                for jlo in range(4):
                    idx = (jg * NW + w) * 4 + jlo
                    x2ps = x2_pool.tile([64, WSZ], F32, tag="x2ps")
                    nc.tensor.matmul(x2ps[:],
                                     w2big[:, jlo * 64:(jlo + 1) * 64],
                                     x1r[:], start=True, stop=True)
                    scr = scr_pool.tile([64, WSZ], F32, tag="scr")
                    nc.scalar.activation(scr[:], x2ps[:], AF.Square,
                                         accum_out=sq2buf[:, idx:idx + 1])
                    scr1 = scr_pool.tile([64, WSZ], F32, tag="scr1")
                    nc.scalar.activation(scr1[:], x2ps[:], AF.Identity,
                                         accum_out=s2buf[:, idx:idx + 1])

    with (tc.tile_pool(name="stp2", bufs=1) as stp_pool,
          tc.tile_pool(name="stps2", bufs=1, space="PSUM") as stps_pool):
        gn_finalize(64, 8, s2buf, sq2buf, G2, G2T, hc2, 3, 4, 5, A2, B2,
                    stp_pool, stps_pool)

    # =================== PHASE P3: recompute + scores ===================
    with (tc.tile_pool(name="x1p3", bufs=2, space="PSUM") as x1_pool,
          tc.tile_pool(name="x2p3", bufs=2, space="PSUM") as x2_pool,
          tc.tile_pool(name="stp3", bufs=2, space="PSUM") as st_pool,
          tc.tile_pool(name="x1r3", bufs=2) as x1r_pool,
          tc.tile_pool(name="x2r3", bufs=2) as x2r_pool):
        for jg in range(8):
            for w in range(NW):
                x1ps = x1_pool.tile([128, WSZ], F32, tag="x1ps")
                conv1(x1ps, jg, w * WSZ)
                x1r = x1r_pool.tile([128, WSZ], F32, tag="x1r")
                nc.scalar.activation(x1r[:], x1ps[:], AF.Relu,
                                     bias=B1[:], scale=A1[:])
                for jlo in range(4):
                    j = jg * 4 + jlo
                    x2ps = x2_pool.tile([64, WSZ], F32, tag="x2ps")
                    nc.tensor.matmul(x2ps[:],
                                     w2big[:, jlo * 64:(jlo + 1) * 64],
                                     x1r[:], start=True, stop=True)
                    x2r = x2r_pool.tile([64, WSZ], F32, tag="x2r")
                    nc.scalar.activation(x2r[:], x2ps[:], AF.Relu,
                                         bias=B2[:], scale=A2[:])
                    NSUB = WSZ // 128
                    stps = st_pool.tile([128, 64 * NSUB], F32, tag="stps")
                    for ts_ in range(NSUB):
                        nc.tensor.transpose(stps[:, ts_ * 64:(ts_ + 1) * 64],
                                            x2r[:, ts_ * 128:(ts_ + 1) * 128],
                                            ident[0:64, 0:64])
                    t0 = w * NSUB
                    nc.vector.tensor_reduce(
                        sap(scall, t0 * K + j, [[K, NSUB]]),
                        sap(stps, 0, [[64, NSUB], [1, 64]]),
                        AX.X, ALU.max)

    # =================== PHASE P4: softmax + aggregate ===================
    with tc.tile_pool(name="fin", bufs=2) as fin_pool:
        for t in range(NT):
            sc = scall[:, t * K:(t + 1) * K]
            rmx = fin_pool.tile([128, 1], F32, tag="rmx")
            nc.vector.tensor_reduce(rmx[:], sc, AX.X, ALU.max)
            nc.vector.tensor_scalar(rmx[:], rmx[:], -1.0, None, ALU.mult)
            e = fin_pool.tile([128, K], F32, tag="e")
            sume = fin_pool.tile([128, 1], F32, tag="sume")
            nc.scalar.activation(e[:], sc, AF.Exp, bias=rmx[:],
                                 accum_out=sume[:])
            nc.vector.reciprocal(sume[:], sume[:])
            wts = fin_pool.tile([128, K], F32, tag="wts")
            nc.vector.tensor_scalar(wts[:], e[:], sume[:], None, ALU.mult)
            w0 = fin_pool.tile([128, S], F32, tag="w0")
            nc.vector.tensor_tensor(w0[:], wts[:, 0:S], maskf[:, 0:S],
                                    ALU.mult)
            w1s = fin_pool.tile([128, S], F32, tag="w1s")
            nc.vector.tensor_tensor(w1s[:],
                                    sap(wts, K - 1, [[-1, S]]),
                                    maskf[:, K:K + S],
                                    ALU.mult)
            outt = fin_pool.tile([128, 3], F32, tag="outt")
            mg = fin_pool.tile([128, 3 * S], F32, tag="mg")
            nc.vector.tensor_tensor(
                mg[:], resi[0][:, t * 3 * S:(t + 1) * 3 * S],
                sap(w0, 0, [[0, 3], [1, S]]), ALU.mult)
            nc.vector.tensor_reduce(outt[:],
                                    sap(mg, 0, [[S, 3], [1, S]]),
                                    AX.X, ALU.add)
            mg2 = fin_pool.tile([128, 3 * S], F32, tag="mg2")
            nc.vector.tensor_tensor(
                mg2[:], resi[1][:, t * 3 * S:(t + 1) * 3 * S],
                sap(w1s, 0, [[0, 3], [1, S]]), ALU.mult)
            ot2 = fin_pool.tile([128, 3], F32, tag="ot2")
            nc.vector.tensor_reduce(ot2[:],
                                    sap(mg2, 0, [[S, 3], [1, S]]),
                                    AX.X, ALU.add)
            nc.vector.tensor_add(outt[:], outt[:], ot2[:])
            nc.vector.tensor_add(outt[:], outt[:],
                                 qxyz[:, t * 4:t * 4 + 3])
            nc.sync.dma_start(
                _ap(out_d, t * 128 * 3, [[3, 128], [1, 3]]), outt[:])
    ctx.close()



# ---------------------------------------------------------------- SPMD entry
_CACHE = {}


def _get_compiled(n_tiles=32):
    if n_tiles not in _CACHE:
        nc = bacc.Bacc("TRN2", target_bir_lowering=False, debug=False,
                       num_devices=8)
        build(nc, n_tiles=n_tiles)
        _CACHE[n_tiles] = nc
    return _CACHE[n_tiles]


def kernel(points0, points1, k, weighted_t, perm, w1, b1, gn1_w, gn1_b,
           w2, b2, gn2_w, gn2_b, _trace=False):
    from concourse.bass_utils import run_bass_kernel_spmd
    args = dict(points0=np.asarray(points0), points1=np.asarray(points1),
                k=int(np.asarray(k)), weighted_t=np.asarray(weighted_t),
                perm=np.asarray(perm), w1=np.asarray(w1), b1=np.asarray(b1),
                gn1_w=np.asarray(gn1_w), gn1_b=np.asarray(gn1_b),
                w2=np.asarray(w2), b2=np.asarray(b2),
                gn2_w=np.asarray(gn2_w), gn2_b=np.asarray(gn2_b))
    assert args["k"] == 32 and args["points0"].shape == (8, 3, 4096)
    in_maps = host_prep(**args, n_tiles=32)
    nc = _get_compiled(32)
    res = run_bass_kernel_spmd(nc, in_maps, core_ids=list(range(8)),
                               trace=_trace)
    out = np.stack([res.results[i]["out"].T for i in range(8)])  # [8,3,4096]
    out = np.ascontiguousarray(out.astype(np.float32))
    if _trace:
        return out, res
    return out

